# revision 8
# baseline (speedup 1.0000x reference)
"""Biaffine scorer kernel for Trainium2 (Bass/Tile), data-parallel over batch
across 8 NeuronCores.

Reference computation (per batch item b):
    h = leaky_relu(state @ head_w + head_b)          # (S, BS)
    t = leaky_relu(state @ tail_w + tail_b)          # (S, BS)
    scores1[x,y,o] = h[x] @ U[o] @ t[y]
    scores2[x,y,o] = Wh.h1[x] + Wt.t1[y] + Ww.wemb[x,y] + cls_b
    out = scores1 + scores2                          # (S, S, O)

v2: full-bf16 dataflow. tolerance is 2e-2 rel; bf16 end-to-end measures
~1.5e-3, and halving every DMA byte matters because the f32 baseline was
DMA-bandwidth-bound (16 engines ~66% busy).

Device-side decomposition per core (4 batch items, processed in 2 pairs
so matmuls stream 512 moving columns):

    h1T/t1T [121, 512]    = Prelu(head_w.T @ stateT, bias) per (b0|b1);
                            bias + the ones-row enter via the ACT bias AP
                            (no K=1 bias matmul), Prelu alpha=0.01 does the
                            leaky in the same PSUM-evacuating ACT op.
    tUT_bb [121, 2560]    : per o, [U(o).T + folds] @ t1_bb -> contiguous
                            (o,y)-blocks, evacuated by plain ACT/DVE copies
                            (the old (y,o) interleave cost 2.3x on ACT).
    out[x, (o,y)]         = h1T.T @ tUT_bb per 512-col chunk; the C table
                            (+cls_b, +width term) rides the PSUM-evacuating
                            add, split DVE / ACT+GpSimd (GpSimd has no PSUM
                            port, so its chunks take an ACT copy first).

Host side packs constants, transposes state, and un-interleaves the
(o,y)-major bf16 output back to (S, S, O) f32.
"""

import numpy as np
import ml_dtypes

import concourse.bass as bass
import concourse.bacc as bacc
import concourse.tile as tile
from concourse import mybir
from concourse.bass_utils import run_bass_kernel_spmd

# problem shape (hardcoded per harness contract)
B, S, H = 32, 255, 1024
BS, WD, O = 120, 20, 10
SP = 256            # padded S
SP2 = 2 * SP        # paired moving dim
NW = SP * O         # 2560
KT = H // 128       # 8
NCORES = 8
BPC = B // NCORES   # 4 batch items per core
NP = BPC // 2       # 2 pairs per core
BSE = BS + 1        # 121

F32 = mybir.dt.float32
BF16 = mybir.dt.bfloat16
NPBF = ml_dtypes.bfloat16

_CACHE: dict = {}


def _emit(tc, d):
    """Emit the per-core program. d: dict of DRAM APs."""
    from contextlib import ExitStack

    nc = tc.nc
    AF = mybir.ActivationFunctionType

    with ExitStack() as ctx:
        const = ctx.enter_context(tc.tile_pool(name="const", bufs=1))
        st_pool = ctx.enter_context(tc.tile_pool(name="st", bufs=2))
        ht_pool = ctx.enter_context(tc.tile_pool(name="ht", bufs=2))
        tut_pool = ctx.enter_context(tc.tile_pool(name="tut", bufs=2))
        out_pool = ctx.enter_context(tc.tile_pool(name="outp", bufs=3))
        # 8 PSUM banks: pp_u [121,1024]x2 (proj + tUT), pp_s [128,1024]x2
        pp_u = ctx.enter_context(tc.tile_pool(name="pp_u", bufs=2, space="PSUM"))
        pp_s = ctx.enter_context(tc.tile_pool(name="pp_s", bufs=2, space="PSUM"))

        # ---- persistent constants (qSP ring, in first-use order) ----
        # weights carry an extra zero column -> psum row 120 = 0; the ACT
        # bias AP then sets row 120 to Prelu(0 + 1.0) = 1.0 (the ones row).
        sb_hw = const.tile([128, KT * BSE], BF16)
        nc.sync.dma_start(sb_hw[:], d["hw"])
        sb_tw = const.tile([128, KT * BSE], BF16)
        nc.sync.dma_start(sb_tw[:], d["tw"])
        # bias: col 0 = head_b (+1.0 at row 120), col 1 = tail_b
        sb_bias = const.tile([BSE, 2], F32)
        nc.sync.dma_start(sb_bias[:], d["bias"])
        # ut: per-o [121, 121] blocks (U.T with Wt in col 120, Wh folded
        # into the ones-row), then 2 spare cols.
        sb_ut = const.tile([BSE, O * BSE + 2], BF16)
        nc.sync.dma_start(sb_ut[:], d["ut"])
        sb_c0 = const.tile([128, NW], BF16)
        nc.sync.dma_start(sb_c0[:], d["cmat"][0:128, :])
        sb_c1 = const.tile([128, NW], BF16)
        nc.sync.dma_start(sb_c1[:], d["cmat"][128:256, :])

        # ---- state loads (qAct ring, all up front; seq is idle then) ----
        half = KT * SP2 // 2
        sb_st = []
        for p in range(NP):
            a = st_pool.tile([128, half], BF16, name="sTa")
            b = st_pool.tile([128, half], BF16, name="sTb")
            nc.scalar.dma_start(a[:], d["stateT"][p][:, 0:half])
            nc.scalar.dma_start(b[:], d["stateT"][p][:, half:])
            sb_st.append((a, b))

        ht1 = [None, None]   # [121, 1024] bf16: h1T cols 0:512, t1T 512:1024
        tUT = [None, None]   # [121, 2, 2560] bf16 per pair

        def proj(p):
            # head/tail projections -> ht1[p], bf16, via Prelu+bias evac
            ps = pp_u.tile([128, 2 * SP2], F32, name="ps_u")[0:BSE]
            for side, w in ((0, sb_hw), (1, sb_tw)):
                for kt in range(KT):
                    st = sb_st[p][0] if kt < 4 else sb_st[p][1]
                    nc.tensor.matmul(
                        ps[:, side * SP2:(side + 1) * SP2],
                        lhsT=w[:, kt * BSE:(kt + 1) * BSE],
                        rhs=st[:, (kt % 4) * SP2:(kt % 4 + 1) * SP2],
                        start=(kt == 0),
                        stop=(kt == KT - 1),
                    )
            ht1[p] = ht_pool.tile([BSE, 2 * SP2], BF16, name="ht1")
            nc.scalar.activation(
                ht1[p][:, 0:SP2], ps[:, 0:SP2], AF.Prelu,
                bias=sb_bias[:, 0:1], alpha=0.01,
            )
            nc.scalar.activation(
                ht1[p][:, SP2:2 * SP2], ps[:, SP2:2 * SP2], AF.Prelu,
                bias=sb_bias[:, 1:2], alpha=0.01,
            )

        def tut_group(p, q, evac):
            # one 1024-wide PSUM tile = o-pair (2q, 2q+1) for both b0 and b1;
            # single evac writes chunk q*512 of both tUT[p][:, bb, :].
            t1T = ht1[p][:, SP2:2 * SP2]
            ps_u = pp_u.tile([128, 1024], F32, name="ps_u")[0:BSE]
            for bb in range(2):
                rhs = t1T[:, bb * SP:(bb + 1) * SP]
                for s in range(2):
                    nc.tensor.matmul(
                        ps_u[:, (2 * bb + s) * SP:(2 * bb + s + 1) * SP],
                        lhsT=sb_ut[:, (2 * q + s) * BSE:(2 * q + s + 1) * BSE],
                        rhs=rhs,
                        start=True,
                        stop=True,
                    )
            dst = tUT[p][:, :, q * 512:(q + 1) * 512]
            if evac == "A":
                nc.scalar.activation(dst, ps_u[:], AF.Copy)
            else:
                nc.vector.tensor_copy(dst, ps_u[:])

        def finals_tile(p, bb, xt, evacs):
            # out[x, (o,y)] for one 128-row x-tile of one batch item.
            # chunks: [1024, 1024, 512]; evacs = 3 engine codes:
            #   D = DVE add; G = ACT copy + gps add; X = ACT copy + DVE add
            sb_c = sb_c0 if xt == 0 else sb_c1
            sb_out = out_pool.tile([128, NW], BF16, name="sb_out")
            lo = bb * SP + xt * 128
            lhsT = ht1[p][:, lo:lo + 128]
            for ci, (c0, w) in enumerate(((0, 1024), (1024, 1024), (2048, 512))):
                ps_s = pp_s.tile([128, 1024], F32, name="ps_s")
                for s in range(w // 512):
                    nc.tensor.matmul(
                        ps_s[:, s * 512:(s + 1) * 512],
                        lhsT=lhsT,
                        rhs=tUT[p][:, bb, c0 + s * 512:c0 + (s + 1) * 512],
                        start=True,
                        stop=True,
                    )
                oc = sb_out[:, c0:c0 + w]
                cc = sb_c[:, c0:c0 + w]
                e = evacs[ci]
                if e == "D":
                    nc.vector.tensor_add(oc, ps_s[:, 0:w], cc)
                elif e == "G":
                    nc.scalar.activation(oc, ps_s[:, 0:w], AF.Copy)
                    nc.gpsimd.tensor_add(oc, oc, cc)
                else:  # "X": ACT evac + DVE bf16 2x-mode add
                    nc.scalar.activation(oc, ps_s[:, 0:w], AF.Copy)
                    nc.vector.tensor_add(oc, oc, cc)
            orow = xt * 128
            nc.sync.dma_start(
                d["out"][2 * p + bb, orow:orow + 64, :], sb_out[0:64, :]
            )
            nc.sync.dma_start(
                d["out"][2 * p + bb, orow + 64:orow + 128, :], sb_out[64:128, :]
            )

        for p in range(NP):
            tUT[p] = tut_pool.tile([BSE, 2, NW], BF16, name="tUT")

        # ---- software pipeline: A0 B0 A1 [C0 x B1] C1 ----
        # engine budget: ACT ~18.8us, DVE ~18.5us, gps ~12.8us, all under
        # the ~30us DMA-engine floor; phases overlap so ACT (tUT p1) and
        # DVE/gps (finals p0) run concurrently.
        proj(0)
        for q, e in enumerate("ADADA"):
            tut_group(0, q, e)
        proj(1)
        # interleave pair-1 tUT (ACT evacs) with pair-0 finals (DVE/gps)
        tut_group(1, 0, "A")
        finals_tile(0, 0, 0, "DDG")
        tut_group(1, 1, "A")
        tut_group(1, 2, "A")
        finals_tile(0, 0, 1, "DDG")
        tut_group(1, 3, "A")
        finals_tile(0, 1, 0, "DDG")
        tut_group(1, 4, "A")
        finals_tile(0, 1, 1, "DDG")
        finals_tile(1, 0, 0, "DGG")
        finals_tile(1, 0, 1, "DXG")
        finals_tile(1, 1, 0, "DXG")
        finals_tile(1, 1, 1, "DGG")


def build_nc():
    if "nc" in _CACHE:
        return _CACHE["nc"]
    nc = bacc.Bacc(
        "TRN2", target_bir_lowering=False, debug=False, num_devices=NCORES
    )
    d = {}
    d["stateT"] = nc.dram_tensor(
        "stateT", [NP, 128, KT * SP2], BF16, kind="ExternalInput"
    ).ap()
    d["hw"] = nc.dram_tensor("hw", [128, KT * BSE], BF16, kind="ExternalInput").ap()
    d["tw"] = nc.dram_tensor("tw", [128, KT * BSE], BF16, kind="ExternalInput").ap()
    d["ut"] = nc.dram_tensor(
        "ut", [BSE, O * BSE + 2], BF16, kind="ExternalInput"
    ).ap()
    d["bias"] = nc.dram_tensor("bias", [BSE, 2], F32, kind="ExternalInput").ap()
    d["cmat"] = nc.dram_tensor("cmat", [SP, NW], BF16, kind="ExternalInput").ap()
    d["out"] = nc.dram_tensor("out", [BPC, SP, NW], BF16, kind="ExternalOutput").ap()

    with tile.TileContext(nc) as tc:
        _emit(tc, d)
    nc.compile()
    _CACHE["nc"] = nc
    return nc


def prep_inputs(inputs):
    """Host-side constant packing + state transpose. Returns dict of np arrays
    shared across cores (stateT is full-batch; shard before dispatch)."""
    state = np.asarray(inputs["state"], np.float32)
    head_w = np.asarray(inputs["head_w"], np.float32)
    head_b = np.asarray(inputs["head_b"], np.float32)
    tail_w = np.asarray(inputs["tail_w"], np.float32)
    tail_b = np.asarray(inputs["tail_b"], np.float32)
    U = np.asarray(inputs["U"], np.float32)
    width_table = np.asarray(inputs["width_table"], np.float32)
    cls_w = np.asarray(inputs["cls_w"], np.float32)
    cls_b = np.asarray(inputs["cls_b"], np.float32)

    # stateT paired pack: [B/2, 128, (kt, b01, y)], y zero-padded to 256
    stateT = np.zeros((B, H, SP), np.float32)
    stateT[:, :, :S] = state.transpose(0, 2, 1)
    stateT = stateT.reshape(B // 2, 2, KT, 128, SP).transpose(0, 3, 2, 1, 4)
    stateT = np.ascontiguousarray(
        stateT.reshape(B // 2, 128, KT * SP2).astype(NPBF)
    )

    hw_sb = np.zeros((128, KT, BSE), np.float32)
    hw_sb[:, :, :BS] = head_w.reshape(KT, 128, BS).transpose(1, 0, 2)
    hw_sb = np.ascontiguousarray(hw_sb.reshape(128, KT * BSE).astype(NPBF))
    tw_sb = np.zeros((128, KT, BSE), np.float32)
    tw_sb[:, :, :BS] = tail_w.reshape(KT, 128, BS).transpose(1, 0, 2)
    tw_sb = np.ascontiguousarray(tw_sb.reshape(128, KT * BSE).astype(NPBF))

    # ut blocks + 2 spare cols
    ut = np.zeros((BSE, O * BSE + 2), np.float32)
    blocks = ut[:, :O * BSE].reshape(BSE, O, BSE)
    blocks[:BS, :, :BS] = U.transpose(2, 0, 1)           # [j, o, i] = U[o,i,j]
    blocks[:, :, BS] = cls_w[:, BS + 1:2 * (BS + 1)].T   # Wt (incl ones coeff)
    # fold the Wh projection (A-term) into the ones-row of each block:
    # t1T row 120 is all-ones, so adding Wh_ext[o, i] here adds A[x, o]
    # (broadcast over y) to the final scores.
    blocks[BS, :, :] += cls_w[:, :BSE]
    ut = np.ascontiguousarray(ut.astype(NPBF))

    bias = np.zeros((BSE, 2), np.float32)
    bias[:BS, 0] = head_b
    bias[BS, 0] = 1.0                                    # ones-row constant
    bias[:BS, 1] = tail_b
    bias[BS, 1] = 1.0

    pos = np.arange(S)[None, :] - np.arange(S)[:, None] + 1
    pos = pos * (pos > 0)
    wproj = width_table @ cls_w[:, 2 * (BS + 1):].T + cls_b   # [256, 10]
    cmat = np.zeros((SP, NW), np.float32)
    # (o, y)-major: C[x, o*256 + y] = wproj[pos(x,y), o]
    cmat[:S, :].reshape(S, O, SP)[:, :, :S] = wproj[pos].transpose(0, 2, 1)
    cmat = np.ascontiguousarray(cmat.astype(NPBF))

    return {
        "stateT": stateT,
        "hw": hw_sb,
        "tw": tw_sb,
        "ut": ut,
        "bias": bias,
        "cmat": cmat,
    }


def run(inputs, trace=False, trace_kwargs=None):
    nc = build_nc()
    full = prep_inputs(inputs)
    shared = {k: v for k, v in full.items() if k != "stateT"}
    in_maps = []
    for c in range(NCORES):
        m = dict(shared)
        m["stateT"] = np.ascontiguousarray(full["stateT"][c * NP:(c + 1) * NP])
        in_maps.append(m)
    res = run_bass_kernel_spmd(
        nc,
        in_maps,
        core_ids=list(range(NCORES)),
        trace=trace,
        **(trace_kwargs or {}),
    )
    out = np.concatenate([r["out"] for r in res.results], axis=0)
    # [B, 256, (o,y)] bf16 -> [B, S, S, O] f32
    out = out.reshape(B, SP, O, SP).astype(np.float32)
    out = np.ascontiguousarray(out[:, :S, :, :S].transpose(0, 1, 3, 2))
    return out, res


def kernel(**inputs):
    out, _ = run(inputs, trace=False)
    return out


if __name__ == "__main__":
    build_nc()
    print("build ok")


# revision 10
# speedup vs baseline: 1.1340x; 1.1340x over previous
"""Biaffine scorer kernel for Trainium2 (Bass/Tile), data-parallel over batch
across 8 NeuronCores.

Reference computation (per batch item b):
    h = leaky_relu(state @ head_w + head_b)          # (S, BS)
    t = leaky_relu(state @ tail_w + tail_b)          # (S, BS)
    scores1[x,y,o] = h[x] @ U[o] @ t[y]
    scores2[x,y,o] = Wh.h1[x] + Wt.t1[y] + Ww.wemb[x,y] + cls_b
    out = scores1 + scores2                          # (S, S, O)

v2: full-bf16 dataflow. tolerance is 2e-2 rel; bf16 end-to-end measures
~1.5e-3, and halving every DMA byte matters because the f32 baseline was
DMA-bandwidth-bound (16 engines ~66% busy).

Device-side decomposition per core (4 batch items, processed in 2 pairs
so matmuls stream 512 moving columns):

    h1T/t1T [121, 512]    = Prelu(head_w.T @ stateT, bias) per (b0|b1);
                            bias + the ones-row enter via the ACT bias AP
                            (no K=1 bias matmul), Prelu alpha=0.01 does the
                            leaky in the same PSUM-evacuating ACT op.
    tUT_bb [121, 2560]    : per o, [U(o).T + folds] @ t1_bb -> contiguous
                            (o,y)-blocks, evacuated by plain ACT/DVE copies
                            (the old (y,o) interleave cost 2.3x on ACT).
    out[x, (o,y)]         = h1T.T @ tUT_bb per 512-col chunk; the C table
                            (+cls_b, +width term) rides the PSUM-evacuating
                            add, split DVE / ACT+GpSimd (GpSimd has no PSUM
                            port, so its chunks take an ACT copy first).

Host side packs constants, transposes state, and un-interleaves the
(o,y)-major bf16 output back to (S, S, O) f32.
"""

import numpy as np
import ml_dtypes

import concourse.bass as bass
import concourse.bacc as bacc
import concourse.tile as tile
from concourse import mybir
from concourse.bass_utils import run_bass_kernel_spmd

# problem shape (hardcoded per harness contract)
B, S, H = 32, 255, 1024
BS, WD, O = 120, 20, 10
SP = 256            # padded S
SP2 = 2 * SP        # paired moving dim
NW = SP * O         # 2560
KT = H // 128       # 8
NCORES = 8
BPC = B // NCORES   # 4 batch items per core
NP = BPC // 2       # 2 pairs per core
BSE = BS + 1        # 121

F32 = mybir.dt.float32
BF16 = mybir.dt.bfloat16
NPBF = ml_dtypes.bfloat16

_CACHE: dict = {}


def _emit(tc, d):
    """Emit the per-core program. d: dict of DRAM APs."""
    from contextlib import ExitStack

    nc = tc.nc
    AF = mybir.ActivationFunctionType

    with ExitStack() as ctx:
        const = ctx.enter_context(tc.tile_pool(name="const", bufs=1))
        st_pool = ctx.enter_context(tc.tile_pool(name="st", bufs=2))
        ht_pool = ctx.enter_context(tc.tile_pool(name="ht", bufs=2))
        tut_pool = ctx.enter_context(tc.tile_pool(name="tut", bufs=2))
        out_pool = ctx.enter_context(tc.tile_pool(name="outp", bufs=3))
        # 8 PSUM banks: pp_u [121,1024]x2 (proj + tUT), pp_s [128,1024]x2
        pp_u = ctx.enter_context(tc.tile_pool(name="pp_u", bufs=2, space="PSUM"))
        pp_s = ctx.enter_context(tc.tile_pool(name="pp_s", bufs=2, space="PSUM"))

        # ---- persistent constants (qSP ring, in first-use order) ----
        # weights carry an extra zero column -> psum row 120 = 0; the ACT
        # bias AP then sets row 120 to Prelu(0 + 1.0) = 1.0 (the ones row).
        sb_hw = const.tile([128, KT * BSE], BF16)
        nc.sync.dma_start(sb_hw[:], d["hw"])
        sb_tw = const.tile([128, KT * BSE], BF16)
        nc.sync.dma_start(sb_tw[:], d["tw"])
        # bias: col 0 = head_b (+1.0 at row 120), col 1 = tail_b
        sb_bias = const.tile([BSE, 2], F32)
        nc.sync.dma_start(sb_bias[:], d["bias"])
        # ut: per-o [121, 121] blocks (U.T with Wt in col 120, Wh folded
        # into the ones-row), then 2 spare cols.
        sb_ut = const.tile([BSE, O * BSE + 2], BF16)
        nc.sync.dma_start(sb_ut[:], d["ut"])
        sb_c0 = const.tile([128, NW], BF16)
        nc.sync.dma_start(sb_c0[:], d["cmat"][0:128, :])
        sb_c1 = const.tile([128, NW], BF16)
        nc.sync.dma_start(sb_c1[:], d["cmat"][128:256, :])

        # ---- state loads (qAct ring, all up front; seq is idle then) ----
        half = KT * SP2 // 2
        sb_st = []
        for p in range(NP):
            a = st_pool.tile([128, half], BF16, name="sTa")
            b = st_pool.tile([128, half], BF16, name="sTb")
            nc.scalar.dma_start(a[:], d["stateT"][p][:, 0:half])
            nc.scalar.dma_start(b[:], d["stateT"][p][:, half:])
            sb_st.append((a, b))

        ht1 = [None, None]   # [121, 1024] bf16: h1T cols 0:512, t1T 512:1024
        tUT = [None, None]   # [121, 2, 2560] bf16 per pair

        def proj(p):
            # head/tail projections -> ht1[p], bf16, via Prelu+bias evac
            ps = pp_u.tile([128, 2 * SP2], F32, name="ps_u")[0:BSE]
            for side, w in ((0, sb_hw), (1, sb_tw)):
                for kt in range(KT):
                    st = sb_st[p][0] if kt < 4 else sb_st[p][1]
                    nc.tensor.matmul(
                        ps[:, side * SP2:(side + 1) * SP2],
                        lhsT=w[:, kt * BSE:(kt + 1) * BSE],
                        rhs=st[:, (kt % 4) * SP2:(kt % 4 + 1) * SP2],
                        start=(kt == 0),
                        stop=(kt == KT - 1),
                    )
            ht1[p] = ht_pool.tile([BSE, 2 * SP2], BF16, name="ht1")
            nc.scalar.activation(
                ht1[p][:, 0:SP2], ps[:, 0:SP2], AF.Prelu,
                bias=sb_bias[:, 0:1], alpha=0.01,
            )
            nc.scalar.activation(
                ht1[p][:, SP2:2 * SP2], ps[:, SP2:2 * SP2], AF.Prelu,
                bias=sb_bias[:, 1:2], alpha=0.01,
            )

        def tut_group(p, q, evac):
            # one 1024-wide PSUM tile = o-pair (2q, 2q+1) for both b0 and b1;
            # single evac writes chunk q*512 of both tUT[p][:, bb, :].
            t1T = ht1[p][:, SP2:2 * SP2]
            ps_u = pp_u.tile([128, 1024], F32, name="ps_u")[0:BSE]
            for bb in range(2):
                rhs = t1T[:, bb * SP:(bb + 1) * SP]
                for s in range(2):
                    nc.tensor.matmul(
                        ps_u[:, (2 * bb + s) * SP:(2 * bb + s + 1) * SP],
                        lhsT=sb_ut[:, (2 * q + s) * BSE:(2 * q + s + 1) * BSE],
                        rhs=rhs,
                        start=True,
                        stop=True,
                    )
            dst = tUT[p][:, :, q * 512:(q + 1) * 512]
            if evac == "A":
                nc.scalar.activation(dst, ps_u[:], AF.Copy)
            else:
                nc.vector.tensor_copy(dst, ps_u[:])

        def finals_tile(p, bb, xt, evacs):
            # out[x, (o,y)] for one 128-row x-tile of one batch item.
            # chunks: [1024, 1024, 512]; evacs = 3 engine codes:
            #   D = DVE add; G = ACT copy + gps add; X = ACT copy + DVE add
            sb_c = sb_c0 if xt == 0 else sb_c1
            sb_out = out_pool.tile([128, NW], BF16, name="sb_out")
            lo = bb * SP + xt * 128
            lhsT = ht1[p][:, lo:lo + 128]
            for ci, (c0, w) in enumerate(((0, 1024), (1024, 1024), (2048, 512))):
                ps_s = pp_s.tile([128, 1024], F32, name="ps_s")
                for s in range(w // 512):
                    nc.tensor.matmul(
                        ps_s[:, s * 512:(s + 1) * 512],
                        lhsT=lhsT,
                        rhs=tUT[p][:, bb, c0 + s * 512:c0 + (s + 1) * 512],
                        start=True,
                        stop=True,
                    )
                oc = sb_out[:, c0:c0 + w]
                cc = sb_c[:, c0:c0 + w]
                e = evacs[ci]
                if e == "D":
                    nc.vector.tensor_add(oc, ps_s[:, 0:w], cc)
                elif e == "G":
                    nc.scalar.activation(oc, ps_s[:, 0:w], AF.Copy)
                    nc.gpsimd.tensor_add(oc, oc, cc)
                else:  # "X": ACT evac + DVE bf16 2x-mode add
                    nc.scalar.activation(oc, ps_s[:, 0:w], AF.Copy)
                    nc.vector.tensor_add(oc, oc, cc)
                # per-chunk output DMA: streams as soon as each chunk's
                # evac lands (128 rows x 2048B descriptors spread fine)
                orow = xt * 128
                nc.sync.dma_start(
                    d["out"][2 * p + bb, orow:orow + 128, c0:c0 + w], oc
                )

        for p in range(NP):
            tUT[p] = tut_pool.tile([BSE, 2, NW], BF16, name="tUT")

        # ---- software pipeline: A0 B0 A1 [C0 x B1] C1 ----
        # engine budget: ACT ~18.8us, DVE ~18.5us, gps ~12.8us, all under
        # the ~30us DMA-engine floor; phases overlap so ACT (tUT p1) and
        # DVE/gps (finals p0) run concurrently.
        # emit order tuned for earliest + smoothest output stream: first
        # finals tile right after B0; proj(1)/tut(1) matmuls fill PE while
        # pair-0 finals evacs drain on DVE/gps; tail tiles use all engines.
        proj(0)
        for q, e in enumerate("ADADA"):
            tut_group(0, q, e)
        finals_tile(0, 0, 0, "DDG")
        proj(1)
        finals_tile(0, 0, 1, "DDG")
        tut_group(1, 0, "A")
        finals_tile(0, 1, 0, "DDG")
        tut_group(1, 1, "A")
        finals_tile(0, 1, 1, "DDG")
        tut_group(1, 2, "A")
        tut_group(1, 3, "A")
        tut_group(1, 4, "A")
        finals_tile(1, 0, 0, "DGG")
        finals_tile(1, 0, 1, "DXG")
        finals_tile(1, 1, 0, "DXG")
        finals_tile(1, 1, 1, "DXG")


def build_nc():
    if "nc" in _CACHE:
        return _CACHE["nc"]
    nc = bacc.Bacc(
        "TRN2", target_bir_lowering=False, debug=False, num_devices=NCORES
    )
    d = {}
    d["stateT"] = nc.dram_tensor(
        "stateT", [NP, 128, KT * SP2], BF16, kind="ExternalInput"
    ).ap()
    d["hw"] = nc.dram_tensor("hw", [128, KT * BSE], BF16, kind="ExternalInput").ap()
    d["tw"] = nc.dram_tensor("tw", [128, KT * BSE], BF16, kind="ExternalInput").ap()
    d["ut"] = nc.dram_tensor(
        "ut", [BSE, O * BSE + 2], BF16, kind="ExternalInput"
    ).ap()
    d["bias"] = nc.dram_tensor("bias", [BSE, 2], F32, kind="ExternalInput").ap()
    d["cmat"] = nc.dram_tensor("cmat", [SP, NW], BF16, kind="ExternalInput").ap()
    d["out"] = nc.dram_tensor("out", [BPC, SP, NW], BF16, kind="ExternalOutput").ap()

    with tile.TileContext(nc) as tc:
        _emit(tc, d)
    nc.compile()
    _CACHE["nc"] = nc
    return nc


def prep_inputs(inputs):
    """Host-side constant packing + state transpose. Returns dict of np arrays
    shared across cores (stateT is full-batch; shard before dispatch)."""
    state = np.asarray(inputs["state"], np.float32)
    head_w = np.asarray(inputs["head_w"], np.float32)
    head_b = np.asarray(inputs["head_b"], np.float32)
    tail_w = np.asarray(inputs["tail_w"], np.float32)
    tail_b = np.asarray(inputs["tail_b"], np.float32)
    U = np.asarray(inputs["U"], np.float32)
    width_table = np.asarray(inputs["width_table"], np.float32)
    cls_w = np.asarray(inputs["cls_w"], np.float32)
    cls_b = np.asarray(inputs["cls_b"], np.float32)

    # stateT paired pack: [B/2, 128, (kt, b01, y)], y zero-padded to 256
    stateT = np.zeros((B, H, SP), np.float32)
    stateT[:, :, :S] = state.transpose(0, 2, 1)
    stateT = stateT.reshape(B // 2, 2, KT, 128, SP).transpose(0, 3, 2, 1, 4)
    stateT = np.ascontiguousarray(
        stateT.reshape(B // 2, 128, KT * SP2).astype(NPBF)
    )

    hw_sb = np.zeros((128, KT, BSE), np.float32)
    hw_sb[:, :, :BS] = head_w.reshape(KT, 128, BS).transpose(1, 0, 2)
    hw_sb = np.ascontiguousarray(hw_sb.reshape(128, KT * BSE).astype(NPBF))
    tw_sb = np.zeros((128, KT, BSE), np.float32)
    tw_sb[:, :, :BS] = tail_w.reshape(KT, 128, BS).transpose(1, 0, 2)
    tw_sb = np.ascontiguousarray(tw_sb.reshape(128, KT * BSE).astype(NPBF))

    # ut blocks + 2 spare cols
    ut = np.zeros((BSE, O * BSE + 2), np.float32)
    blocks = ut[:, :O * BSE].reshape(BSE, O, BSE)
    blocks[:BS, :, :BS] = U.transpose(2, 0, 1)           # [j, o, i] = U[o,i,j]
    blocks[:, :, BS] = cls_w[:, BS + 1:2 * (BS + 1)].T   # Wt (incl ones coeff)
    # fold the Wh projection (A-term) into the ones-row of each block:
    # t1T row 120 is all-ones, so adding Wh_ext[o, i] here adds A[x, o]
    # (broadcast over y) to the final scores.
    blocks[BS, :, :] += cls_w[:, :BSE]
    ut = np.ascontiguousarray(ut.astype(NPBF))

    bias = np.zeros((BSE, 2), np.float32)
    bias[:BS, 0] = head_b
    bias[BS, 0] = 1.0                                    # ones-row constant
    bias[:BS, 1] = tail_b
    bias[BS, 1] = 1.0

    pos = np.arange(S)[None, :] - np.arange(S)[:, None] + 1
    pos = pos * (pos > 0)
    wproj = width_table @ cls_w[:, 2 * (BS + 1):].T + cls_b   # [256, 10]
    cmat = np.zeros((SP, NW), np.float32)
    # (o, y)-major: C[x, o*256 + y] = wproj[pos(x,y), o]
    cmat[:S, :].reshape(S, O, SP)[:, :, :S] = wproj[pos].transpose(0, 2, 1)
    cmat = np.ascontiguousarray(cmat.astype(NPBF))

    return {
        "stateT": stateT,
        "hw": hw_sb,
        "tw": tw_sb,
        "ut": ut,
        "bias": bias,
        "cmat": cmat,
    }


def run(inputs, trace=False, trace_kwargs=None):
    nc = build_nc()
    full = prep_inputs(inputs)
    shared = {k: v for k, v in full.items() if k != "stateT"}
    in_maps = []
    for c in range(NCORES):
        m = dict(shared)
        m["stateT"] = np.ascontiguousarray(full["stateT"][c * NP:(c + 1) * NP])
        in_maps.append(m)
    res = run_bass_kernel_spmd(
        nc,
        in_maps,
        core_ids=list(range(NCORES)),
        trace=trace,
        **(trace_kwargs or {}),
    )
    out = np.concatenate([r["out"] for r in res.results], axis=0)
    # [B, 256, (o,y)] bf16 -> [B, S, S, O] f32
    out = out.reshape(B, SP, O, SP).astype(np.float32)
    out = np.ascontiguousarray(out[:, :S, :, :S].transpose(0, 1, 3, 2))
    return out, res


def kernel(**inputs):
    out, _ = run(inputs, trace=False)
    return out


if __name__ == "__main__":
    build_nc()
    print("build ok")
